# revision 6
# baseline (speedup 1.0000x reference)
"""Expert-parallel grouped GEMM (MoE) kernel for Trainium2.

Problem: inputs [65536, 1024] sorted by expert (8192 tokens/expert),
weight [8, 512, 1024]; out[t] = x[t] @ W[expert(t)].T -> [65536, 512].

Sharding: expert-parallel across 8 NeuronCores. Tokens are already sorted
by expert and expert_size is static, so core e simply takes token rows
[e*8192:(e+1)*8192] and weight[e] - no all-to-all needed.

Device kernel (per core): one [8192,1024] @ [1024,512] GEMM.
- Both matmul operands need the contraction dim (I) on the SBUF partition
  axis, so the host pre-transposes x -> xT [I, S] and W[e] -> wT [I, O].
- Hybrid fp8/fp16 split-K: the first KT8*128 contraction indices run as
  fp8-e4m3 matmuls with perf_mode=DoubleRow (2 fp8 weights/cell -> 2
  MACs/cell/cycle, 2x the fp16 FLOP rate; contraction 256 per matmul via
  paired k-tiles), the remaining indices run as fp16 matmuls (1
  cycle/row). Both accumulate into the same fp32 PSUM bank. e4m3's 3-bit
  mantissa gives ~2.7% rms error per operand; putting fraction f of the
  contraction in fp8 yields output error ~3.75%*sqrt(f), so KT8=2
  (f=1/4) lands at ~1.88e-2 Frobenius rel err vs the 2e-2 gate while
  cutting PE time by ~KT8/16.
- w is pre-scaled by 64 before e4m3 quantization (keeps w ~N(0,1/32^2)
  out of the e4m3 subnormal range, where relative error degrades); the
  fp16 weights carry the same x64 so the PSUM is uniformly scaled, and
  the PSUM->SBUF output copy multiplies by 1/64 (exact powers of 2, zero
  added error, same DVE cost as the plain copy).
- x stationary (xstat): the PE's stationary operand is the x tile and
  consecutive matmuls stream weight columns through it, so per-matmul
  LDWEIGHTS (128 cols fp16 / 256 cols fp8-pair) hides under the previous
  512-cycle moving stream.
- Outputs leave as fp16 [S, O] (native layout, no host transpose); xT
  streams in prefetched blocks on the SP HWDGE ring; outputs leave on
  the ACT HWDGE ring.
"""

import numpy as np

E = 8          # experts == cores
O = 512        # out_features
I = 1024       # in_features
S = 8192       # tokens per expert
KT = I // 128  # k-tiles (contraction)
OT = O // 128  # o-tiles
SC = 512       # tokens per matmul (moving free dim), wstat only
S_BLK = 2048   # max tokens per streamed x block
BLOCKS = (512, 1536, 2048, 2048, 1536, 512)  # ramp up AND down, sums to S
X_BUFS = 4     # x block buffers (prefetch depth)
DEDUP = False  # strip redundant LDWEIGHTS post-compile (no win measured)
OUT_B = 4      # t-tiles batched per output DMA
KT8 = 2        # leading k-tiles (of 128) done in fp8-e4m3 DoubleRow; even
S_W = 64.0     # w pre-scale before quantization (exact power of 2)

_cache = {}


def _merge_sync(mybir, inst, waits, updates):
    si = inst.sync_info
    if si is None:
        inst.sync_info = mybir.SyncInfo(on_wait=list(waits), on_update=list(updates))
    else:
        si.on_wait = list(waits) + list(si.on_wait)
        si.on_update = list(si.on_update) + list(updates)


def _dedup_ldweights(nc):
    """Remove InstLdweights that reload the identical weight tile.

    Tracks the last-loaded weight signature along each block's PE stream;
    resets at any PE instruction other than a plain matmul (branches,
    drains, barriers, transposes), so loop back-edges stay conservative.
    Waits/updates of removed loads move to the next kept PE instruction.
    """
    from concourse import mybir

    removed = 0
    for fn in nc.m.functions:
        for blk in fn.blocks:
            insts = blk.instructions
            keep = []
            last_sig = None
            pend_w, pend_u = [], []
            for inst in insts:
                if inst.engine != mybir.EngineType.PE:
                    keep.append(inst)
                    continue
                if isinstance(inst, mybir.InstLdweights) and not inst.is_transpose:
                    a = inst.ins[0]
                    sig = (a.memref, a.offset, str(a.ap),
                           str(inst.tile_position), str(inst.perf_mode))
                    if sig == last_sig:
                        si = inst.sync_info
                        if si is not None:
                            pend_w.extend(list(si.on_wait))
                            pend_u.extend(list(si.on_update))
                        removed += 1
                        continue
                    last_sig = sig
                elif not (isinstance(inst, mybir.InstMatmult)
                          and not inst.is_transpose):
                    last_sig = None
                if pend_w or pend_u:
                    _merge_sync(mybir, inst, pend_w, pend_u)
                    pend_w, pend_u = [], []
                keep.append(inst)
            assert not pend_w and not pend_u, "dangling sync from removed ldweights"
            insts[:] = keep
    return removed


def _build_nc(repeats=1, loop=0, idle=0, kt8=KT8):
    import concourse.bass as bass
    import concourse.tile as tile
    from concourse import bacc, mybir
    from contextlib import nullcontext

    kt16 = KT - kt8
    assert kt8 % 2 == 0 and 0 <= kt8 <= KT
    blocks = []  # (start_token, n_tokens)
    pos = 0
    for sz in BLOCKS:
        blocks.append((pos, sz))
        pos += sz
    assert pos == S and all(sz <= S_BLK for _, sz in blocks)

    nc = bacc.Bacc("TRN2", target_bir_lowering=False, debug=False)
    xT8 = wT8 = xT16 = wT16 = None
    if kt8:
        xT8 = nc.dram_tensor("xT8", [kt8 * 128, S], mybir.dt.float8e4,
                             kind="ExternalInput")
        wT8 = nc.dram_tensor("wT8", [kt8 * 128, O], mybir.dt.float8e4,
                             kind="ExternalInput")
    if kt16:
        xT16 = nc.dram_tensor("xT16", [kt16 * 128, S], mybir.dt.float16,
                              kind="ExternalInput")
        wT16 = nc.dram_tensor("wT16", [kt16 * 128, O], mybir.dt.float16,
                              kind="ExternalInput")
    outT = nc.dram_tensor("out", [S, O], mybir.dt.float16, kind="ExternalOutput")
    if idle:
        ping = nc.dram_tensor("ping", [1, 8], mybir.dt.float16)
        pong = nc.dram_tensor("pong", [1, 8], mybir.dt.float16)

    with tile.TileContext(nc) as tc:
        with (
            tc.tile_pool(name="wpool", bufs=1) as wpool,
            tc.tile_pool(name="xpool", bufs=X_BUFS) as xpool,
            tc.tile_pool(name="opool", bufs=4) as opool,
            tc.tile_pool(name="psum", bufs=8, space=bass.MemorySpace.PSUM) as psum_pool,
        ):
            wt8 = (wpool.tile([128, kt8, O], mybir.dt.float8e4, name="wt8")
                   if kt8 else None)
            wt16 = (wpool.tile([128, kt16, O], mybir.dt.float16, name="wt16")
                    if kt16 else None)

            def load_block(blk, with_weights=False):
                # with_weights: interleave the resident-weight k-tile loads
                # with this block's stripes so the first matmul (needs only
                # wt[k=0] + stripe[k=0]) starts ~5us earlier than with a
                # serial full-weight prefix.
                s0, sz = blk
                xblk8 = xblk16 = None
                if kt8:
                    xblk8 = xpool.tile([128, kt8, sz], mybir.dt.float8e4,
                                       name="xblk8", tag="xblk8")
                    for k in range(kt8):
                        if with_weights:
                            nc.sync.dma_start(wt8[:, k, :],
                                              wT8[k * 128:(k + 1) * 128, :])
                        nc.sync.dma_start(
                            xblk8[:, k, :],
                            xT8[k * 128:(k + 1) * 128, s0:s0 + sz],
                        )
                if kt16:
                    xblk16 = xpool.tile([128, kt16, sz], mybir.dt.float16,
                                        name="xblk16", tag="xblk16")
                    for k in range(kt16):
                        if with_weights:
                            nc.sync.dma_start(wt16[:, k, :],
                                              wT16[k * 128:(k + 1) * 128, :])
                        nc.sync.dma_start(
                            xblk16[:, k, :],
                            xT16[k * 128:(k + 1) * 128, s0:s0 + sz],
                        )
                return xblk8, xblk16

            last_ot = [None]

            def compute_block(blk, xblks):
                xblk8, xblk16 = xblks
                s0, sz = blk
                n_t = sz // 128
                for tg in range((n_t + OUT_B - 1) // OUT_B):
                    tb = min(OUT_B, n_t - tg * OUT_B)
                    ot = opool.tile([128, tb, O], mybir.dt.float16, tag="ot")
                    for ti in range(tb):
                        t = tg * OUT_B + ti
                        ps = psum_pool.tile([128, O], mybir.dt.float32,
                                            name="ps", tag="ps")
                        for kp in range(kt8 // 2):
                            nc.tensor.matmul(
                                ps[:],
                                xblk8[:, 2 * kp:2 * kp + 2,
                                      t * 128:(t + 1) * 128],
                                wt8[:, 2 * kp:2 * kp + 2, :],
                                start=(kp == 0),
                                stop=(kp == kt8 // 2 - 1 and kt16 == 0),
                                perf_mode=mybir.MatmulPerfMode.DoubleRow,
                            )
                        for k in range(kt16):
                            nc.tensor.matmul(
                                ps[:],
                                xblk16[:, k, t * 128:(t + 1) * 128],
                                wt16[:, k, :],
                                start=(kt8 == 0 and k == 0),
                                stop=(k == kt16 - 1),
                            )
                        if kt8:
                            nc.vector.tensor_scalar_mul(
                                ot[:, ti, :], ps[:], 1.0 / S_W)
                        else:
                            nc.vector.tensor_copy(ot[:, ti, :], ps[:])
                    g0 = s0 + tg * OUT_B * 128
                    dst = outT[g0:g0 + tb * 128, :].rearrange(
                        "(t p) o -> p t o", p=128)
                    nc.scalar.dma_start(dst, ot[:])
                    last_ot[0] = ot[:, 0, :]

            loop_cm = (
                tc.For_i(0, loop, 1,
                         hint_engines=(mybir.EngineType.PE, mybir.EngineType.SP,
                                       mybir.EngineType.DVE))
                if loop else nullcontext()
            )
            with loop_cm:
                for _ in range(repeats):
                    pending = []  # (blk, xblks) loaded but not yet computed
                    for bi, blk in enumerate(blocks):
                        pending.append((blk, load_block(blk, with_weights=bi == 0)))
                        if len(pending) >= X_BUFS:
                            compute_block(*pending.pop(0))
                    for blk, xblks in pending:
                        compute_block(blk, xblks)
                # low-power idle: dependent tiny DMA ping-pong through one
                # SBUF tile (Tile tracks the tile's RAW/WAR deps, so the
                # copies serialize on each other's completion latency).
                # The first copy reads the gemm's final output tile, so the
                # idle runs strictly AFTER the gemm instead of alongside it,
                # and the per-iteration span is gemm_span + idle_span.
                # Keeps average chip power low so duty-cycled benchmarks see
                # the unthrottled PE clock.
                if idle:
                    idle_t = wpool.tile([1, 8], mybir.dt.float16, name="idle_t")
                    if last_ot[0] is not None:
                        nc.sync.dma_start(idle_t[:], last_ot[0][0:1, 0:8])
                    for i in range(idle):
                        if i % 2 == 0:
                            nc.sync.dma_start(pong[:], idle_t[:])
                        else:
                            nc.sync.dma_start(idle_t[:], ping[:])
    nc.compile()
    if DEDUP and repeats > 0:
        _dedup_ldweights(nc)
    return nc


def _get_nc(repeats=1, loop=0, idle=0, kt8=KT8):
    key = (repeats, loop, idle, kt8, BLOCKS, X_BUFS, DEDUP, OUT_B)
    if key not in _cache:
        _cache[key] = _build_nc(repeats, loop, idle, kt8)
    return _cache[key]


def prep_in_maps(inputs, weight, kt8=KT8):
    """Per-core input tensors: transposed, split by dtype along K."""
    import ml_dtypes

    i8 = kt8 * 128
    f8 = ml_dtypes.float8_e4m3  # TRN FP8_EXP4: IEEE-style, max +-240
    in_maps = []
    for e in range(E):
        xT = inputs[e * S:(e + 1) * S, :].T  # [I, S]
        wT = weight[e].T  # [I, O]
        m = {}
        if kt8:
            m["xT8"] = np.ascontiguousarray(
                np.clip(xT[:i8], -240, 240)).astype(f8)
            m["wT8"] = np.ascontiguousarray(
                np.clip(wT[:i8] * np.float32(S_W), -240, 240)).astype(f8)
        if kt8 < KT:
            m["xT16"] = np.ascontiguousarray(xT[i8:]).astype(np.float16)
            w16 = wT[i8:] * np.float32(S_W) if kt8 else wT[i8:]
            m["wT16"] = np.ascontiguousarray(w16).astype(np.float16)
        in_maps.append(m)
    return in_maps


def run(inputs, weight, trace=False, repeats=1, loop=0, kt8=KT8):
    """Shard, run on 8 cores, gather. Returns (out, BassKernelResults)."""
    from concourse.bass_utils import run_bass_kernel_spmd

    nc = _get_nc(repeats, loop, kt8=kt8)
    in_maps = prep_in_maps(inputs, weight, kt8=kt8)
    res = run_bass_kernel_spmd(nc, in_maps, list(range(E)), trace=trace)
    outs = [res.results[e]["out"] for e in range(E)]
    out = np.concatenate([o.astype(np.float32) for o in outs], axis=0)
    return out, res


def kernel(inputs, weight, expert_size):
    inputs = np.asarray(inputs, dtype=np.float32)
    weight = np.asarray(weight, dtype=np.float32)
    assert inputs.shape == (E * S, I) and weight.shape == (E, O, I)
    assert int(expert_size) == S
    out, _ = run(inputs, weight, trace=False)
    return out


# revision 13
# speedup vs baseline: 1.3494x; 1.3494x over previous
"""Expert-parallel grouped GEMM (MoE) kernel for Trainium2.

Problem: inputs [65536, 1024] sorted by expert (8192 tokens/expert),
weight [8, 512, 1024]; out[t] = x[t] @ W[expert(t)].T -> [65536, 512].

Sharding: expert-parallel across 8 NeuronCores. Tokens are already sorted
by expert and expert_size is static, so core e simply takes token rows
[e*8192:(e+1)*8192] and weight[e] - no all-to-all needed.

Device kernel (per core): one [8192,1024] @ [1024,512] GEMM.
- Both matmul operands need the contraction dim (I) on the SBUF partition
  axis, so the host pre-transposes x -> xT [I, S] and W[e] -> wT [I, O].
- Hybrid fp8/fp16 split-K: the first KT8*128 contraction indices run as
  fp8-e4m3 matmuls with perf_mode=DoubleRow (2 fp8 weights/cell -> 2
  MACs/cell/cycle, 2x the fp16 FLOP rate; contraction 256 per matmul via
  paired k-tiles), the remaining indices run as fp16 matmuls (1
  cycle/row). Both accumulate into the same fp32 PSUM bank. e4m3's 3-bit
  mantissa gives ~2.7% rms error per operand; putting fraction f of the
  contraction in fp8 yields output error ~3.75%*sqrt(f), so KT8=2
  (f=1/4) lands at ~1.88e-2 Frobenius rel err vs the 2e-2 gate while
  cutting PE time by ~KT8/16.
- w is pre-scaled by 64 before e4m3 quantization (keeps w ~N(0,1/32^2)
  out of the e4m3 subnormal range, where relative error degrades); the
  fp16 weights carry the same x64 so the PSUM is uniformly scaled, and
  the PSUM->SBUF output copy multiplies by 1/64 (exact powers of 2, zero
  added error, same DVE cost as the plain copy).
- x stationary (xstat): the PE's stationary operand is the x tile and
  consecutive matmuls stream weight columns through it, so per-matmul
  LDWEIGHTS (128 cols fp16 / 256 cols fp8-pair) hides under the previous
  512-cycle moving stream. Measured on HW: fp16 matmuls run at exactly
  512 cycles (LDWEIGHTS fully hidden); the DoubleRow matmul costs ~828
  cycles when following another DR matmul (its 256-col LDWEIGHTS cannot
  overlap a DR stream - the DR moving pair occupies the weight XBUS).
- Outputs leave as fp16 [S, O] (native layout, no host transpose); xT
  streams in prefetched blocks on the SP HWDGE ring; outputs leave on
  the ACT HWDGE ring.
"""

import numpy as np

E = 8          # experts == cores
O = 512        # out_features
I = 1024       # in_features
S = 8192       # tokens per expert
KT = I // 128  # k-tiles (contraction)
OT = O // 128  # o-tiles
SC = 512       # tokens per matmul (moving free dim), wstat only
S_BLK = 2048   # max tokens per streamed x block
BLOCKS = (512, 1536, 2048, 2048, 1536, 512)  # ramp up AND down, sums to S
X_BUFS = 4     # x block buffers (prefetch depth)
DEDUP = False  # strip redundant LDWEIGHTS post-compile (no win measured)
OUT_B = 4      # t-tiles batched per output DMA
KT8 = 2        # leading k-tiles (of 128) done in fp8-e4m3 DoubleRow; even
S_W = 64.0     # w pre-scale before quantization (exact power of 2)
DR_LAST = False  # emit the DoubleRow matmul after the fp16 ones per t-tile

_cache = {}


def _merge_sync(mybir, inst, waits, updates):
    si = inst.sync_info
    if si is None:
        inst.sync_info = mybir.SyncInfo(on_wait=list(waits), on_update=list(updates))
    else:
        si.on_wait = list(waits) + list(si.on_wait)
        si.on_update = list(si.on_update) + list(updates)


def _dedup_ldweights(nc):
    """Remove InstLdweights that reload the identical weight tile.

    Tracks the last-loaded weight signature along each block's PE stream;
    resets at any PE instruction other than a plain matmul (branches,
    drains, barriers, transposes), so loop back-edges stay conservative.
    Waits/updates of removed loads move to the next kept PE instruction.
    """
    from concourse import mybir

    removed = 0
    for fn in nc.m.functions:
        for blk in fn.blocks:
            insts = blk.instructions
            keep = []
            last_sig = None
            pend_w, pend_u = [], []
            for inst in insts:
                if inst.engine != mybir.EngineType.PE:
                    keep.append(inst)
                    continue
                if isinstance(inst, mybir.InstLdweights) and not inst.is_transpose:
                    a = inst.ins[0]
                    sig = (a.memref, a.offset, str(a.ap),
                           str(inst.tile_position), str(inst.perf_mode))
                    if sig == last_sig:
                        si = inst.sync_info
                        if si is not None:
                            pend_w.extend(list(si.on_wait))
                            pend_u.extend(list(si.on_update))
                        removed += 1
                        continue
                    last_sig = sig
                elif not (isinstance(inst, mybir.InstMatmult)
                          and not inst.is_transpose):
                    last_sig = None
                if pend_w or pend_u:
                    _merge_sync(mybir, inst, pend_w, pend_u)
                    pend_w, pend_u = [], []
                keep.append(inst)
            assert not pend_w and not pend_u, "dangling sync from removed ldweights"
            insts[:] = keep
    return removed


def _build_nc(repeats=1, loop=0, idle=0, kt8=KT8):
    import concourse.bass as bass
    import concourse.tile as tile
    from concourse import bacc, mybir
    from contextlib import nullcontext

    kt16 = KT - kt8
    assert kt8 % 2 == 0 and 0 <= kt8 <= KT
    blocks = []  # (start_token, n_tokens)
    pos = 0
    for sz in BLOCKS:
        blocks.append((pos, sz))
        pos += sz
    assert pos == S and all(sz <= S_BLK for _, sz in blocks)

    nc = bacc.Bacc("TRN2", target_bir_lowering=False, debug=False)
    xT8 = wT8 = xT16 = wT16 = None
    if kt8:
        xT8 = nc.dram_tensor("xT8", [kt8 * 128, S], mybir.dt.float8e4,
                             kind="ExternalInput")
        wT8 = nc.dram_tensor("wT8", [kt8 * 128, O], mybir.dt.float8e4,
                             kind="ExternalInput")
    if kt16:
        xT16 = nc.dram_tensor("xT16", [kt16 * 128, S], mybir.dt.float16,
                              kind="ExternalInput")
        wT16 = nc.dram_tensor("wT16", [kt16 * 128, O], mybir.dt.float16,
                              kind="ExternalInput")
    outT = nc.dram_tensor("out", [S, O], mybir.dt.float16, kind="ExternalOutput")
    if idle:
        ping = nc.dram_tensor("ping", [1, 8], mybir.dt.float16)
        pong = nc.dram_tensor("pong", [1, 8], mybir.dt.float16)

    with tile.TileContext(nc) as tc:
        with (
            tc.tile_pool(name="wpool", bufs=1) as wpool,
            tc.tile_pool(name="xpool", bufs=X_BUFS) as xpool,
            tc.tile_pool(name="opool", bufs=4) as opool,
            tc.tile_pool(name="psum", bufs=8, space=bass.MemorySpace.PSUM) as psum_pool,
        ):
            wt8 = (wpool.tile([128, kt8, O], mybir.dt.float8e4, name="wt8")
                   if kt8 else None)
            wt16 = (wpool.tile([128, kt16, O], mybir.dt.float16, name="wt16")
                    if kt16 else None)

            def load_block(blk, with_weights=False):
                # with_weights: interleave the resident-weight k-tile loads
                # with this block's stripes so the first matmul (needs only
                # wt[k=0] + stripe[k=0]) starts ~5us earlier than with a
                # serial full-weight prefix.
                s0, sz = blk
                xblk8 = xblk16 = None
                if kt8:
                    xblk8 = xpool.tile([128, kt8, sz], mybir.dt.float8e4,
                                       name="xblk8", tag="xblk8")
                    for k in range(kt8):
                        if with_weights:
                            nc.sync.dma_start(wt8[:, k, :],
                                              wT8[k * 128:(k + 1) * 128, :])
                        nc.sync.dma_start(
                            xblk8[:, k, :],
                            xT8[k * 128:(k + 1) * 128, s0:s0 + sz],
                        )
                if kt16:
                    xblk16 = xpool.tile([128, kt16, sz], mybir.dt.float16,
                                        name="xblk16", tag="xblk16")
                    for k in range(kt16):
                        if with_weights:
                            nc.sync.dma_start(wt16[:, k, :],
                                              wT16[k * 128:(k + 1) * 128, :])
                        nc.sync.dma_start(
                            xblk16[:, k, :],
                            xT16[k * 128:(k + 1) * 128, s0:s0 + sz],
                        )
                return xblk8, xblk16

            last_ot = [None]

            def compute_block(blk, xblks):
                xblk8, xblk16 = xblks
                s0, sz = blk
                n_t = sz // 128
                for tg in range((n_t + OUT_B - 1) // OUT_B):
                    tb = min(OUT_B, n_t - tg * OUT_B)
                    ot = opool.tile([128, tb, O], mybir.dt.float16, tag="ot")
                    for ti in range(tb):
                        t = tg * OUT_B + ti
                        ps = psum_pool.tile([128, O], mybir.dt.float32,
                                            name="ps", tag="ps")
                        # DR_LAST: fp16 matmuls first, fp8 DoubleRow last;
                        # the PSUM accumulation chain then forces each DR
                        # matmul after this tile's fp16 matmuls, so its
                        # serial 256-col LDWEIGHTS can hide under a
                        # preceding fp16 matmul stream.
                        def emit_dr(first, last):
                            for kp in range(kt8 // 2):
                                nc.tensor.matmul(
                                    ps[:],
                                    xblk8[:, 2 * kp:2 * kp + 2,
                                          t * 128:(t + 1) * 128],
                                    wt8[:, 2 * kp:2 * kp + 2, :],
                                    start=(first and kp == 0),
                                    stop=(last and kp == kt8 // 2 - 1),
                                    perf_mode=mybir.MatmulPerfMode.DoubleRow,
                                )

                        def emit_f16(first, last):
                            for k in range(kt16):
                                nc.tensor.matmul(
                                    ps[:],
                                    xblk16[:, k, t * 128:(t + 1) * 128],
                                    wt16[:, k, :],
                                    start=(first and k == 0),
                                    stop=(last and k == kt16 - 1),
                                )

                        if DR_LAST:
                            emit_f16(True, kt8 == 0)
                            emit_dr(kt16 == 0, True)
                        else:
                            emit_dr(True, kt16 == 0)
                            emit_f16(kt8 == 0, True)
                        if kt8:
                            nc.vector.tensor_scalar_mul(
                                ot[:, ti, :], ps[:], 1.0 / S_W)
                        else:
                            nc.vector.tensor_copy(ot[:, ti, :], ps[:])
                    g0 = s0 + tg * OUT_B * 128
                    dst = outT[g0:g0 + tb * 128, :].rearrange(
                        "(t p) o -> p t o", p=128)
                    nc.scalar.dma_start(dst, ot[:])
                    last_ot[0] = ot[:, 0, :]

            loop_cm = (
                tc.For_i(0, loop, 1,
                         hint_engines=(mybir.EngineType.PE, mybir.EngineType.SP,
                                       mybir.EngineType.DVE))
                if loop else nullcontext()
            )
            with loop_cm:
                for _ in range(repeats):
                    pending = []  # (blk, xblks) loaded but not yet computed
                    for bi, blk in enumerate(blocks):
                        pending.append((blk, load_block(blk, with_weights=bi == 0)))
                        if len(pending) >= X_BUFS:
                            compute_block(*pending.pop(0))
                    for blk, xblks in pending:
                        compute_block(blk, xblks)
                # low-power idle: dependent tiny DMA ping-pong through one
                # SBUF tile (Tile tracks the tile's RAW/WAR deps, so the
                # copies serialize on each other's completion latency).
                # The first copy reads the gemm's final output tile, so the
                # idle runs strictly AFTER the gemm instead of alongside it,
                # and the per-iteration span is gemm_span + idle_span.
                # Keeps average chip power low so duty-cycled benchmarks see
                # the unthrottled PE clock.
                if idle:
                    idle_t = wpool.tile([1, 8], mybir.dt.float16, name="idle_t")
                    if last_ot[0] is not None:
                        nc.sync.dma_start(idle_t[:], last_ot[0][0:1, 0:8])
                    for i in range(idle):
                        if i % 2 == 0:
                            nc.sync.dma_start(pong[:], idle_t[:])
                        else:
                            nc.sync.dma_start(idle_t[:], ping[:])
    nc.compile()
    if DEDUP and repeats > 0:
        _dedup_ldweights(nc)
    return nc


def _get_nc(repeats=1, loop=0, idle=0, kt8=KT8):
    key = (repeats, loop, idle, kt8, BLOCKS, X_BUFS, DEDUP, OUT_B, DR_LAST, S_W)
    if key not in _cache:
        _cache[key] = _build_nc(repeats, loop, idle, kt8)
    return _cache[key]


def prep_in_maps(inputs, weight, kt8=KT8):
    """Per-core input tensors: transposed, split by dtype along K."""
    import ml_dtypes

    i8 = kt8 * 128
    f8 = ml_dtypes.float8_e4m3  # TRN FP8_EXP4: IEEE-style, max +-240
    in_maps = []
    for e in range(E):
        xT = inputs[e * S:(e + 1) * S, :].T  # [I, S]
        wT = weight[e].T  # [I, O]
        m = {}
        if kt8:
            m["xT8"] = np.ascontiguousarray(
                np.clip(xT[:i8], -240, 240)).astype(f8)
            m["wT8"] = np.ascontiguousarray(
                np.clip(wT[:i8] * np.float32(S_W), -240, 240)).astype(f8)
        if kt8 < KT:
            m["xT16"] = np.ascontiguousarray(xT[i8:]).astype(np.float16)
            w16 = wT[i8:] * np.float32(S_W) if kt8 else wT[i8:]
            m["wT16"] = np.ascontiguousarray(w16).astype(np.float16)
        in_maps.append(m)
    return in_maps


def run(inputs, weight, trace=False, repeats=1, loop=0, kt8=KT8):
    """Shard, run on 8 cores, gather. Returns (out, BassKernelResults)."""
    from concourse.bass_utils import run_bass_kernel_spmd

    nc = _get_nc(repeats, loop, kt8=kt8)
    in_maps = prep_in_maps(inputs, weight, kt8=kt8)
    res = run_bass_kernel_spmd(nc, in_maps, list(range(E)), trace=trace)
    outs = [res.results[e]["out"] for e in range(E)]
    out = np.concatenate([o.astype(np.float32) for o in outs], axis=0)
    return out, res


def kernel(inputs, weight, expert_size):
    inputs = np.asarray(inputs, dtype=np.float32)
    weight = np.asarray(weight, dtype=np.float32)
    assert inputs.shape == (E * S, I) and weight.shape == (E, O, I)
    assert int(expert_size) == S
    out, _ = run(inputs, weight, trace=False)
    return out
